# revision 1
# baseline (speedup 1.0000x reference)
"""GCN layer kernel for trn2: host prep + bass kernel builder + runner."""
import sys
sys.path.insert(0, '/opt/trn_rl_repo')
import numpy as np
import ml_dtypes
from dataclasses import dataclass

import concourse.bass as bass
import concourse.bacc as bacc
import concourse.mybir as mybir
import concourse.tile as tile
from concourse.bass_utils import run_bass_kernel_spmd

F32 = mybir.dt.float32
BF16 = mybir.dt.bfloat16
I16 = mybir.dt.int16
U32 = mybir.dt.uint32
FP8 = mybir.dt.float8e4
AF = mybir.ActivationFunctionType
OP = mybir.AluOpType


@dataclass
class Cfg:
    nodes: int            # padded node count (mult of 128*cores)
    e_real: int
    lsplit: int           # nodes < lsplit go to L table
    cores: int = 8
    d: int = 128
    kcut: int = 0
    nlmax: int = 0        # L chunks per tile
    nhmax: int = 0        # H chunks per tile
    cand_cols: int = 256
    nrounds_local: int = 4
    phases: str = "full"   # "build", "A", "T", "full"

    @property
    def tpc(self):
        return self.nodes // 128 // self.cores

    @property
    def own(self):
        return self.nodes // self.cores

    @property
    def nct(self):
        return self.nlmax + self.nhmax

    @property
    def nchunk(self):
        return self.tpc * self.nct

    @property
    def slots(self):
        return self.nchunk * 128

    @property
    def ntiles_all(self):
        return self.nodes // 128

    @property
    def zl(self):
        return self.lsplit

    @property
    def zh(self):
        return self.nodes - self.lsplit

    @property
    def hrows(self):
        return self.nodes - self.lsplit + 1


def host_prep(cfg: Cfg, src, dst):
    E = src.shape[0]
    core_of = dst // cfg.own
    out = []
    for c in range(cfg.cores):
        sel = np.nonzero(core_of == c)[0]
        s = src[sel].astype(np.int64)
        dloc_all = dst[sel].astype(np.int64) - c * cfg.own
        t_all = dloc_all // 128
        loc_all = dloc_all % 128
        is_h = (s >= cfg.lsplit).astype(np.int64)
        gidx = np.zeros(cfg.slots, np.int16)
        dstloc = np.full(cfg.slots, -1.0, np.float32)
        order = np.lexsort((loc_all, is_h, t_all))
        s, t_all, loc_all, is_h = s[order], t_all[order], loc_all[order], is_h[order]
        for t in range(cfg.tpc):
            base_slot = t * cfg.nct * 128
            m = t_all == t
            for hs, budget, off in ((0, cfg.nlmax, 0), (1, cfg.nhmax, cfg.nlmax * 128)):
                mm = m & (is_h == hs)
                n = int(mm.sum())
                assert n <= budget * 128, f"core{c} tile{t} hs{hs}: {n}>{budget*128}"
                sl = slice(base_slot + off, base_slot + off + budget * 128)
                gi = np.full(budget * 128, cfg.zl if hs == 0 else cfg.zh, np.int64)
                gi[:n] = s[mm] - (cfg.lsplit if hs else 0)
                gidx[sl] = gi.astype(np.int16)
                dl = np.full(budget * 128, -1.0, np.float32)
                dl[:n] = loc_all[mm].astype(np.float32)
                dstloc[sl] = dl
        gidx_w = np.ascontiguousarray(
            np.tile(gidx.reshape(-1, 16).T, (8, 1)))            # [128, slots/16]
        dstloc_pc = np.ascontiguousarray(
            dstloc.reshape(cfg.nchunk, 128).T)                  # [128, nchunk]
        sexp = (dstloc.reshape(1, -1) ==
                np.arange(128, dtype=np.float32).reshape(128, 1))
        sexp = sexp.astype(ml_dtypes.float8_e4m3)               # [128, slots]
        out.append(dict(gidx=gidx_w, dstloc=dstloc_pc, sexp=sexp))
    return out


def build_nc(cfg: Cfg):
    nc = bacc.Bacc(None)
    d = cfg.d
    TPC, NCT, NCH = cfg.tpc, cfg.nct, cfg.nchunk
    NL128, NH128 = cfg.nlmax * 128, cfg.nhmax * 128
    KCUT = float(cfg.kcut)

    feat = nc.dram_tensor("feat", [cfg.nodes, d], F32, kind="ExternalInput")
    wT = nc.dram_tensor("wT", [d, d], F32, kind="ExternalInput")
    hpre = nc.dram_tensor("hpre", [cfg.own, d], F32, kind="ExternalInput")
    degp = nc.dram_tensor("degp", [128, TPC], F32, kind="ExternalInput")
    degall = nc.dram_tensor("degall", [128, cfg.ntiles_all], F32, kind="ExternalInput")
    gidx_e = nc.dram_tensor("gidx", [128, cfg.slots // 16], I16, kind="ExternalInput")
    dstloc_e = nc.dram_tensor("dstloc", [128, NCH], F32, kind="ExternalInput")
    sexp_e = nc.dram_tensor("sexp", [128, cfg.slots], FP8, kind="ExternalInput")
    h_ext = nc.dram_tensor("h", [cfg.own, d], F32, kind="ExternalOutput")
    cos_dbg = nc.dram_tensor("cos_dbg", [128, NCH], F32, kind="ExternalOutput")
    ts_dbg = nc.dram_tensor("ts_dbg", [1, 4], F32, kind="ExternalOutput")
    cand_dbg = nc.dram_tensor("cand_dbg", [128, cfg.cand_cols], F32, kind="ExternalOutput")
    gcnt_dbg = nc.dram_tensor("gcnt_dbg", [8, 32], F32, kind="ExternalOutput")

    cc_in = nc.dram_tensor("cc_in", [1, 32], F32)
    cc_out = nc.dram_tensor("cc_out", [1, 32], F32, addr_space="Shared")
    ag_in = nc.dram_tensor("ag_in", [16, cfg.cand_cols], F32)
    ag_out = nc.dram_tensor("ag_out", [cfg.cores, 16, cfg.cand_cols], F32,
                            addr_space="Shared")
    groups = [list(range(cfg.cores))]

    with tile.TileContext(nc) as tc:
        with (tc.tile_pool(name="const", bufs=1) as cpool,
              tc.tile_pool(name="state", bufs=1) as spool,
              tc.tile_pool(name="dram", bufs=1, space="DRAM") as dpool,
              tc.tile_pool(name="gath", bufs=2) as gpool,
              tc.tile_pool(name="ftile", bufs=3) as fpool,
              tc.tile_pool(name="ypsum", bufs=2, space="PSUM") as ypool,
              tc.tile_pool(name="apsum", bufs=2, space="PSUM") as apool,
              tc.tile_pool(name="misc", bufs=2) as mpool,
              tc.tile_pool(name="thr", bufs=1) as tpool):

            # ---------- DRAM tables ----------
            nhL = dpool.tile([cfg.lsplit + 1, d], F32, tag="nhL")
            nhH = dpool.tile([cfg.hrows, d], F32, tag="nhH")
            ftL = dpool.tile([cfg.lsplit + 1, d], F32, tag="ftL")
            ftH = dpool.tile([cfg.hrows, d], F32, tag="ftH")

            # ---------- constants / inputs to SBUF ----------
            iota_row = cpool.tile([128, 128], F32, tag="iota_row")
            nc.gpsimd.iota(iota_row[:], pattern=[[1, 128]], base=0,
                           channel_multiplier=0,
                           allow_small_or_imprecise_dtypes=True)
            ones_col = cpool.tile([128, 1], F32, tag="ones_col")
            nc.vector.memset(ones_col[:], 1.0)
            ones_row = cpool.tile([1, 128], F32, tag="ones_row")
            nc.vector.memset(ones_row[:], 1.0)
            zrow = cpool.tile([1, d], F32, tag="zrow")
            nc.vector.memset(zrow[:], 0.0)
            wT_sb = cpool.tile([d, d], F32, tag="wT")
            nc.sync.dma_start(wT_sb[:], wT[:])
            hp_sb = spool.tile([128, TPC * d], F32, tag="hp")
            nc.sync.dma_start(hp_sb[:].rearrange("p (t x) -> p t x", x=d),
                              hpre[:].rearrange("(t p) x -> p t x", p=128))
            degp_sb = spool.tile([128, TPC], F32, tag="degp")
            nc.sync.dma_start(degp_sb[:], degp[:])
            degall_sb = spool.tile([128, cfg.ntiles_all], F32, tag="degall")
            nc.sync.dma_start(degall_sb[:], degall[:])
            gidx_sb = spool.tile([128, cfg.slots // 16], I16, tag="gidx")
            nc.sync.dma_start(gidx_sb[:], gidx_e[:])
            dstloc_sb = spool.tile([128, NCH], F32, tag="dstloc")
            nc.sync.dma_start(dstloc_sb[:], dstloc_e[:])

            # norms
            norm_own = spool.tile([128, TPC], F32, tag="norm_own")
            nc.vector.tensor_scalar_max(norm_own[:], degp_sb[:], 1.0)
            nc.scalar.activation(norm_own[:], norm_own[:], AF.Sqrt)
            nc.vector.reciprocal(norm_own[:], norm_own[:])
            norm_all = spool.tile([128, cfg.ntiles_all], F32, tag="norm_all")
            nc.vector.tensor_scalar_max(norm_all[:], degall_sb[:], 1.0)
            nc.scalar.activation(norm_all[:], norm_all[:], AF.Sqrt)
            nc.vector.reciprocal(norm_all[:], norm_all[:])

            # NHI stationaries (own nh tiles)
            ssq = spool.tile([128, TPC], F32, tag="ssq")
            for t in range(TPC):
                scr = mpool.tile([128, d], F32, tag="sqscr")
                nc.scalar.activation(scr[:], hp_sb[:, t * d:(t + 1) * d],
                                     AF.Square, accum_out=ssq[:, t:t + 1])
            invl_own = spool.tile([128, TPC], F32, tag="invl_own")
            nc.vector.tensor_scalar_max(ssq[:], ssq[:], 1e-24)
            nc.scalar.activation(invl_own[:], ssq[:], AF.Sqrt)
            nc.vector.reciprocal(invl_own[:], invl_own[:])

            # ---------- table build ----------
            for ot in range(cfg.ntiles_all):
                r0 = ot * 128
                ft = fpool.tile([128, d], F32, tag="bf")
                nc.sync.dma_start(ft[:], feat[r0:r0 + 128, :])
                sq = mpool.tile([128, d], F32, tag="bsq")
                acc = mpool.tile([128, 1], F32, tag="bacc")
                nc.scalar.activation(sq[:], ft[:], AF.Square, accum_out=acc[:])
                nc.vector.tensor_scalar_max(acc[:], acc[:], 1e-24)
                nc.scalar.activation(acc[:], acc[:], AF.Sqrt)
                nc.vector.reciprocal(acc[:], acc[:])
                nh_t = fpool.tile([128, d], F32, tag="bnh")
                nc.vector.tensor_scalar_mul(nh_t[:], ft[:], acc[:])
                ft_t = fpool.tile([128, d], F32, tag="bft")
                nc.vector.tensor_scalar_mul(ft_t[:], ft[:], norm_all[:, ot:ot + 1])
                if r0 + 128 <= cfg.lsplit:
                    nc.sync.dma_start(nhL[r0:r0 + 128, :], nh_t[:])
                    nc.sync.dma_start(ftL[r0:r0 + 128, :], ft_t[:])
                elif r0 >= cfg.lsplit:
                    q = r0 - cfg.lsplit
                    nc.sync.dma_start(nhH[q:q + 128, :], nh_t[:])
                    nc.sync.dma_start(ftH[q:q + 128, :], ft_t[:])
                else:
                    ns = cfg.lsplit - r0
                    nc.sync.dma_start(nhL[r0:cfg.lsplit, :], nh_t[:ns, :])
                    nc.sync.dma_start(nhH[0:128 - ns, :], nh_t[ns:, :])
                    nc.sync.dma_start(ftL[r0:cfg.lsplit, :], ft_t[:ns, :])
                    nc.sync.dma_start(ftH[0:128 - ns, :], ft_t[ns:, :])
            nc.sync.dma_start(nhL[cfg.zl:cfg.zl + 1, :], zrow[:])
            nc.sync.dma_start(ftL[cfg.zl:cfg.zl + 1, :], zrow[:])
            nc.sync.dma_start(nhH[cfg.zh:cfg.zh + 1, :], zrow[:])
            nc.sync.dma_start(ftH[cfg.zh:cfg.zh + 1, :], zrow[:])

            # ---------- Phase A: cos ----------
            run_a = cfg.phases in ("A", "T", "full")
            run_t = cfg.phases in ("T", "full")
            run_b = cfg.phases == "full"
            cos_sb = spool.tile([128, NCH], F32, tag="cos")
            if not run_a:
                nc.vector.memset(cos_sb[:, :1], 0.0)
            if run_a:
                for t in range(TPC):
                    nhi_t = mpool.tile([128, d], F32, tag="nhit")
                    nc.vector.tensor_scalar_mul(nhi_t[:],
                                                hp_sb[:, t * d:(t + 1) * d],
                                                invl_own[:, t:t + 1])
                    x_t = gpool.tile([128, NCT, d], F32, tag="x")
                    i0 = t * NCT * 8
                    nc.gpsimd.dma_gather(
                        out_ap=x_t[:, :cfg.nlmax, :], in_ap=nhL[:],
                        idxs_ap=gidx_sb[:, i0:i0 + cfg.nlmax * 8],
                        num_idxs=NL128, num_idxs_reg=NL128, elem_size=d,
                        single_packet=False)
                    nc.gpsimd.dma_gather(
                        out_ap=x_t[:, cfg.nlmax:, :], in_ap=nhH[:],
                        idxs_ap=gidx_sb[:, i0 + cfg.nlmax * 8:i0 + NCT * 8],
                        num_idxs=NH128, num_idxs_reg=NH128, elem_size=d,
                        single_packet=False)
                    se8 = gpool.tile([128, NCT * 128], FP8, tag="se8")
                    nc.sync.dma_start(se8[:],
                                      sexp_e[:, t * NCT * 128:(t + 1) * NCT * 128])
                    se32 = gpool.tile([128, NCT * 128], F32, tag="se32")
                    nc.scalar.copy(se32[:], se8[:])
                    for c in range(NCT):
                        y_ps = ypool.tile([128, d], F32, tag="y")
                        nc.tensor.matmul(y_ps[:], se32[:, c * 128:(c + 1) * 128],
                                         nhi_t[:], start=True, stop=True)
                        scr = mpool.tile([128, d], F32, tag="cscr")
                        cc = t * NCT + c
                        nc.vector.scalar_tensor_tensor(
                            scr[:], x_t[:, c, :], 1.0, y_ps[:],
                            op0=OP.mult, op1=OP.mult,
                            accum_out=cos_sb[:, cc:cc + 1])

            if run_a and not run_t:
                nc.sync.dma_start(cos_dbg[:], cos_sb[:])
            # ---------- Phase T: threshold ----------
            if run_t:
                lo_t = tpool.tile([1, 1], F32, tag="lo")
                th_row = tpool.tile([1, 32], F32, tag="throw")
                th_bc = tpool.tile([128, 32], F32, tag="thbc")
                cnt128 = tpool.tile([128, 32], F32, tag="cnt128")
                gcnt = tpool.tile([1, 32], F32, tag="gcnt")
                srow = tpool.tile([1, 1], F32, tag="srow")
                cbase = tpool.tile([1, 1], F32, tag="cbase")
                iota32 = tpool.tile([1, 32], F32, tag="iota32")
                nc.vector.tensor_copy(iota32[:], iota_row[:1, :32])
                msk = tpool.tile([1, 32], F32, tag="msk")
                msct = tpool.tile([1, 32], F32, tag="msct")
                cscr2 = tpool.tile([128, max(NCH, cfg.cand_cols)], F32, tag="cscr2")
                cand = tpool.tile([128, cfg.cand_cols], F32, tag="cand")
                nc.vector.memset(cbase[:], 0.0)
                nc.vector.memset(lo_t[:], -0.75)

                def emit_round(vals_ap, ncols, w_bin, mode, shift4):
                    # thresholds
                    nc.vector.tensor_scalar_mul(th_row[:], iota32[:], w_bin)
                    nc.vector.tensor_scalar(th_row[:], th_row[:], lo_t[:], None,
                                            op0=OP.add)
                    if shift4:
                        nc.vector.tensor_scalar_add(th_row[:], th_row[:], 4.0)
                    ps = ypool.tile([128, 32], F32, tag="tiny")
                    nc.tensor.matmul(ps[:], ones_row[:], th_row[:],
                                     start=True, stop=True)
                    nc.vector.tensor_copy(th_bc[:], ps[:])
                    for j in range(32):
                        nc.vector.tensor_scalar(
                            cscr2[:, :ncols], vals_ap, th_bc[:, j:j + 1], None,
                            op0=OP.is_lt, op1=OP.add,
                            accum_out=cnt128[:, j:j + 1])
                    cps = ypool.tile([1, 32], F32, tag="tiny")
                    nc.tensor.matmul(cps[:], ones_col[:], cnt128[:],
                                     start=True, stop=True)
                    nc.vector.tensor_copy(gcnt[:], cps[:])
                    nc.sync.dma_start(gcnt_dbg[emit_round.i:emit_round.i+1, :], gcnt[:])
                    emit_round.i += 1
                    if mode.startswith("global"):
                        nc.sync.dma_start(cc_in[:], gcnt[:])
                        nc.gpsimd.collective_compute(
                            "AllReduce", OP.add, replica_groups=groups,
                            ins=[cc_in[:]], outs=[cc_out[:]])
                        nc.sync.dma_start(gcnt[:], cc_out[:])
                    # s = #((gcnt + cbase) < k - .5); sel = max(s-1, 0)
                    nc.vector.tensor_scalar(
                        msct[:], gcnt[:], cbase[:], KCUT - 0.5,
                        op0=OP.add, op1=OP.is_lt)
                    nc.vector.tensor_scalar(
                        msct[:], msct[:], 0.0, None,
                        op0=OP.add, op1=OP.add, accum_out=srow[:])
                    nc.vector.tensor_scalar(srow[:], srow[:], -1.0, 0.0,
                                            op0=OP.add, op1=OP.max)
                    if mode == "global2":
                        nc.vector.tensor_scalar(msk[:], iota32[:], srow[:], None,
                                                op0=OP.is_equal)
                        nc.vector.scalar_tensor_tensor(
                            msct[:], gcnt[:], 1.0, msk[:],
                            op0=OP.mult, op1=OP.mult, accum_out=cbase[:])
                    # lo += sel * w_bin
                    nc.vector.scalar_tensor_tensor(
                        lo_t[:], srow[:], w_bin, lo_t[:], op0=OP.mult, op1=OP.add)

                emit_round.i = 0
                W1 = 1.5 / 32
                W2 = 1.5 / 32 ** 2
                emit_round(cos_sb[:], NCH, W1, "global1", False)
                emit_round(cos_sb[:], NCH, W2, "global2", False)

                # compact in-bracket values, remapped to cos+4
                lo_bc = tpool.tile([128, 1], F32, tag="lobc")
                psb = ypool.tile([128, 1], F32, tag="tiny")
                nc.tensor.matmul(psb[:], ones_row[:], lo_t[:], start=True, stop=True)
                nc.vector.tensor_copy(lo_bc[:], psb[:])
                m1 = tpool.tile([128, NCH], F32, tag="m1")
                nc.vector.tensor_scalar(m1[:], cos_sb[:], lo_bc[:], None, op0=OP.is_ge)
                hi_bc = tpool.tile([128, 1], F32, tag="hibc")
                nc.vector.tensor_scalar_add(hi_bc[:], lo_bc[:], W2)
                m2 = tpool.tile([128, NCH], F32, tag="m2")
                nc.vector.tensor_scalar(m2[:], cos_sb[:], hi_bc[:], None, op0=OP.is_lt)
                nc.vector.tensor_mul(m1[:], m1[:], m2[:])
                c4 = tpool.tile([128, NCH], F32, tag="c4")
                nc.vector.tensor_scalar(c4[:], cos_sb[:], 5.0, None, op0=OP.add)
                # y = m*(cos+5) - 1: in-bracket -> cos+4 (>0), outside -> -1
                nc.vector.tensor_mul(c4[:], c4[:], m1[:])
                nc.vector.tensor_scalar_add(c4[:], c4[:], -1.0)
                y16 = tpool.tile([16, NCH * 8], F32, tag="m2")
                for g in range(8):
                    nc.sync.dma_start(y16[:, g * NCH:(g + 1) * NCH],
                                      c4[16 * g:16 * (g + 1), :])
                NSG = 8
                sg_in_cols = (NCH * 8 + NSG - 1) // NSG
                sg_out_cols = cfg.cand_cols // NSG
                sgc = tpool.tile([16, cfg.cand_cols], F32, tag="sgc")
                posi = tpool.tile([16, sg_out_cols], F32, tag="posi")
                nc.gpsimd.iota(posi[:], pattern=[[16, sg_out_cols]], base=0,
                               channel_multiplier=1,
                               allow_small_or_imprecise_dtypes=True)
                for sg_i in range(NSG):
                    c0 = sg_i * sg_in_cols
                    c1 = min((sg_i + 1) * sg_in_cols, NCH * 8)
                    sg_out = tpool.tile([16, sg_out_cols], F32, tag="sgout")
                    nfound = tpool.tile([1, 1], U32, tag="nfound")
                    nc.gpsimd.sparse_gather(sg_out[:], y16[:, c0:c1],
                                            num_found=nfound[:])
                    nf_f = tpool.tile([1, 1], F32, tag="nff")
                    nc.vector.tensor_copy(nf_f[:], nfound[:])
                    nf16 = tpool.tile([16, 1], F32, tag="nf16")
                    ps16 = ypool.tile([16, 1], F32, tag="tiny")
                    nc.tensor.matmul(ps16[:], ones_row[:, :16], nf_f[:],
                                     start=True, stop=True)
                    nc.vector.tensor_copy(nf16[:], ps16[:])
                    mtail = tpool.tile([16, sg_out_cols], F32, tag="mtail")
                    nc.vector.tensor_scalar(mtail[:], posi[:], nf16[:], None,
                                            op0=OP.is_lt)
                    big = tpool.tile([16, sg_out_cols], F32, tag="big")
                    nc.vector.tensor_scalar(big[:], mtail[:], 0.5, 1e30,
                                            op0=OP.is_lt, op1=OP.mult)
                    nc.vector.tensor_mul(sg_out[:], sg_out[:], mtail[:])
                    nc.vector.tensor_add(
                        sgc[:, sg_i * sg_out_cols:(sg_i + 1) * sg_out_cols],
                        sg_out[:], big[:])
                nc.sync.dma_start(ag_in[:], sgc[:])
                nc.gpsimd.collective_compute(
                    "AllGather", OP.bypass, replica_groups=groups,
                    ins=[ag_in[:]], outs=[ag_out[:]])
                for r in range(cfg.cores):
                    nc.sync.dma_start(cand[16 * r:16 * (r + 1), :], ag_out[r, :, :])

                wr = W2
                for r in range(cfg.nrounds_local):
                    wr = wr / 32
                    emit_round(cand[:], cfg.cand_cols, wr, "local", True)
                nc.vector.tensor_scalar_add(lo_t[:], lo_t[:], wr)  # t* = hi edge
                tstar = tpool.tile([128, 1], F32, tag="tstar")
                pst = ypool.tile([128, 1], F32, tag="tiny")
                nc.tensor.matmul(pst[:], ones_row[:], lo_t[:], start=True, stop=True)
                nc.vector.tensor_copy(tstar[:], pst[:])

                nc.sync.dma_start(cos_dbg[:], cos_sb[:])
                nc.sync.dma_start(ts_dbg[:, 0:1], lo_t[:])
                nc.sync.dma_start(ts_dbg[:, 1:2], cbase[:])
                nc.sync.dma_start(ts_dbg[:, 2:3], srow[:])
                nc.sync.dma_start(ts_dbg[:, 3:4], nf_f[:])
                nc.sync.dma_start(cand_dbg[:], cand[:])
                # dm = keep*(dstloc+1) - 1  (keep = cos >= t*)
                keep = tpool.tile([128, NCH], F32, tag="m1")
                nc.vector.tensor_scalar(keep[:], cos_sb[:], tstar[:], None,
                                        op0=OP.is_ge)
                dm = tpool.tile([128, NCH], F32, tag="c4")
                nc.vector.tensor_scalar_add(dm[:], dstloc_sb[:], 1.0)
                nc.vector.tensor_mul(dm[:], dm[:], keep[:])
                nc.vector.tensor_scalar_add(dm[:], dm[:], -1.0)


            # ---------- Phase B: aggregate + linear + tail ----------
            if run_b:
                for t in range(TPC):
                    xf_t = gpool.tile([128, NCT, d], F32, tag="x")
                    i0 = t * NCT * 8
                    nc.gpsimd.dma_gather(
                        out_ap=xf_t[:, :cfg.nlmax, :], in_ap=ftL[:],
                        idxs_ap=gidx_sb[:, i0:i0 + cfg.nlmax * 8],
                        num_idxs=NL128, num_idxs_reg=NL128, elem_size=d,
                        single_packet=False)
                    nc.gpsimd.dma_gather(
                        out_ap=xf_t[:, cfg.nlmax:, :], in_ap=ftH[:],
                        idxs_ap=gidx_sb[:, i0 + cfg.nlmax * 8:i0 + NCT * 8],
                        num_idxs=NH128, num_idxs_reg=NH128, elem_size=d,
                        single_packet=False)
                    at_ps = apool.tile([128, 128], F32, tag="aggT")
                    for c in range(NCT):
                        sa = mpool.tile([128, 128], F32, tag="sa")
                        cc = t * NCT + c
                        nc.vector.tensor_scalar(sa[:], iota_row[:],
                                                dm[:, cc:cc + 1], None,
                                                op0=OP.is_equal)
                        nc.tensor.matmul(at_ps[:], xf_t[:, c, :], sa[:],
                                         start=(c == 0), stop=(c == NCT - 1))
                    at_sb = mpool.tile([128, 128], F32, tag="aggTsb")
                    nc.scalar.copy(at_sb[:], at_ps[:])
                    h_ps = apool.tile([128, d], F32, tag="hps")
                    nc.tensor.matmul(h_ps[:], at_sb[:], wT_sb[:],
                                     start=True, stop=True)
                    hre = mpool.tile([128, d], F32, tag="hre")
                    nc.scalar.activation(hre[:], h_ps[:], AF.Relu,
                                         scale=norm_own[:, t:t + 1])
                    hout = mpool.tile([128, d], F32, tag="hout")
                    nc.vector.tensor_add(hout[:], hre[:], hp_sb[:, t * d:(t + 1) * d])
                    nc.sync.dma_start(
                        h_ext[:].rearrange("(t p) x -> p t x", p=128)[:, t, :],
                        hout[:])


    nc.finalize()
    return nc


def make_cfg(nodes_pad, lsplit, src, dst, kcut, cores=8):
    cfg = Cfg(nodes=nodes_pad, e_real=len(src), lsplit=lsplit,
              cores=cores, kcut=kcut)
    own = cfg.own
    core_of = dst // own
    nl, nh = 1, 1
    for c in range(cores):
        sel = core_of == c
        s, dd = src[sel], dst[sel]
        t_all = (dd.astype(np.int64) - c * own) // 128
        is_h = s >= lsplit
        for t in range(cfg.tpc):
            m = t_all == t
            nl = max(nl, int(np.ceil((m & ~is_h).sum() / 128)))
            nh = max(nh, int(np.ceil((m & is_h).sum() / 128)))
    cfg.nlmax, cfg.nhmax = nl, nh
    NSG = 8
    sg_in = (cfg.nchunk * 8 + NSG - 1) // NSG
    cfg.cand_cols = NSG * min(64, sg_in)
    return cfg


def make_inputs(cfg: Cfg, features, W, src, dst):
    nreal = features.shape[0]
    featp = np.zeros((cfg.nodes, cfg.d), np.float32)
    featp[:nreal] = features
    deg = np.bincount(dst, minlength=cfg.nodes).astype(np.float32)
    percore = host_prep(cfg, src, dst)
    in_maps = []
    for c in range(cfg.cores):
        base = c * cfg.own
        degp = np.ascontiguousarray(deg[base:base + cfg.own].reshape(cfg.tpc, 128).T)
        degall = np.ascontiguousarray(deg.reshape(cfg.ntiles_all, 128).T)
        pc = percore[c]
        in_maps.append(dict(
            feat=featp, wT=np.ascontiguousarray(W.T).astype(np.float32),
            hpre=np.ascontiguousarray(featp[base:base + cfg.own]),
            degp=degp, degall=degall,
            gidx=pc["gidx"], dstloc=pc["dstloc"], sexp=pc["sexp"]))
    return in_maps


def run(cfg: Cfg, features, W, src, dst):
    in_maps = make_inputs(cfg, features, W, src, dst)
    nc = build_nc(cfg)
    r = run_bass_kernel_spmd(nc, in_maps, core_ids=list(range(cfg.cores)))
    h = np.concatenate([r.results[c]["h"] for c in range(cfg.cores)], axis=0)
    return h[:features.shape[0]]


# ---------------- harness entry point ----------------
def kernel(features, W, src, dst):
    """Full inputs in, full output out. Edges sharded by dst range across
    8 NeuronCores; cosine cut threshold found exactly on-device via
    multi-round counting + candidate compaction + allgather."""
    src = np.asarray(src).astype(np.int32)
    dst = np.asarray(dst).astype(np.int32)
    features = np.asarray(features, dtype=np.float32)
    W = np.asarray(W, dtype=np.float32)
    kcut = int(src.shape[0] * 0.1)
    cfg = make_cfg(50176, 32767, src, dst, kcut)
    return run(cfg, features, W, src, dst).astype(np.float32)

